# revision 11
# baseline (speedup 1.0000x reference)
"""TRN2 Bass kernel for nn_CompleteObservationLearner (loss_fn).

Computes, for E=4096 environment embeddings and Q=16384 queries (D=512):
    proj = relu(env @ w1 + b1) @ w2 + b2
    d2[e,q] = ||proj[e] - query[q]||^2  (via GEMM trick; clamp never triggers)
    predictions = argmin_e d2          loss = CE(-d2, targets)   acc = mean(pred==t)

Distribution (8 NeuronCores):
  - queries sharded across cores (2048/core), data-parallel
  - environment projection sharded (512 env rows/core) then AllGathered
  - scalar loss/accuracy partial sums combined on host

Math notes:
  - softmax over e of (-d2) equals softmax of L[e,q] = 2*proj[e].q - ||proj[e]||^2
    (the ||q||^2 term is constant per column and cancels; the max(...,0) clamp never
    fires for this data distribution: min d2 ~ 507).
  - Main GEMM on PE in float32r (inputs rounded to 11 mantissa bits) with a 2-pass
    hi/lo split => fp32-grade accuracy at 2 cycles/row (native fp32 is 4).
  - -pn is folded into the PSUM accumulation as two rank-1 (K=1) f32r matmuls.
  - Per q-tile softmax offset C0 = max over the first e-chunk; exact
    lse = C0 + log(sum exp(L - C0)); argmax tracked per chunk then combined.
"""

import numpy as np

E, Q, D = 4096, 16384, 512
NCORES = 8
QS = Q // NCORES          # 2048 queries per core
ES = E // NCORES          # 512 env rows per core
NQT = QS // 128           # 16 query tiles per core
CHUNK = 1024              # e-chunk width processed per psum tile
NCH = E // CHUNK          # 4 chunks
NKT = D // 128            # 4 contraction sub-tiles
ROWS = 2 * D + 2          # AllGather rows per core: projT hi, projT lo, -pn hi, -pn lo

_CACHE = {}


def _build(debug=False):
    from contextlib import ExitStack
    import concourse.bass as bass
    import concourse.tile as tile
    from concourse import bacc, mybir

    f32 = mybir.dt.float32
    f32r = mybir.dt.float32r
    i32 = mybir.dt.int32
    AF = mybir.ActivationFunctionType
    ALU = mybir.AluOpType
    AX = mybir.AxisListType
    ts = bass.ts

    nc = bacc.Bacc(None, target_bir_lowering=False, debug=False, num_devices=NCORES)

    # ---- I/O ----
    q_in = nc.dram_tensor("q_shard", [QS, D], f32, kind="ExternalInput")
    env_in = nc.dram_tensor("env_shard", [ES, D], f32, kind="ExternalInput")
    w1_in = nc.dram_tensor("w1", [D, D], f32, kind="ExternalInput")
    w2_in = nc.dram_tensor("w2", [D, D], f32, kind="ExternalInput")
    b1_in = nc.dram_tensor("b1t", [128, NKT], f32, kind="ExternalInput")
    b2_in = nc.dram_tensor("b2t", [128, NKT], f32, kind="ExternalInput")
    t_in = nc.dram_tensor("t_shard", [128, NQT], f32, kind="ExternalInput")

    pred_out = nc.dram_tensor("pred", [128, NQT], i32, kind="ExternalOutput")
    part_out = nc.dram_tensor("partials", [2, 1], f32, kind="ExternalOutput")
    dbg_out = None
    if debug:
        dbg_out = nc.dram_tensor("dbg", [128, 6 * NQT], f32, kind="ExternalOutput")

    # internal DRAM for the projection AllGather
    ag_in = nc.dram_tensor("ag_in", [ROWS, D], f32r)
    ag_out = nc.dram_tensor("ag_out", [NCORES * ROWS, D], f32r, addr_space="Shared")

    with tile.TileContext(nc) as tc, ExitStack() as ctx:
        # ---- pools ----
        # mem: bufs=1, tag-shared slots; phase-local tiles reuse slots across phases
        mem = ctx.enter_context(tc.tile_pool(name="mem", bufs=1))
        work = ctx.enter_context(tc.tile_pool(name="work", bufs=2))
        proj_pool = ctx.enter_context(tc.tile_pool(name="proj", bufs=2))
        npn_pool = ctx.enter_context(tc.tile_pool(name="npn", bufs=1))
        ps_mm = ctx.enter_context(tc.tile_pool(name="psmm", bufs=3, space="PSUM"))
        ps_tp = ctx.enter_context(tc.tile_pool(name="pstp", bufs=2, space="PSUM"))

        # ---- persistent tiles (alive whole kernel) ----
        qT_hi = mem.tile([128, NKT, QS], f32r, tag="qThi")   # round_f32r(2*query^T) [d, ktile, q]
        t_all = mem.tile([128, NQT], f32, tag="t_all")
        stats_m = mem.tile([128, NQT * NCH], f32, tag="st_m")
        stats_i = mem.tile([128, NQT * NCH], f32, tag="st_i")
        stats_g = mem.tile([128, NQT * NCH], f32, tag="st_g")
        stats_s = mem.tile([128, NQT * NCH], f32, tag="st_s")
        negC0 = mem.tile([128, NQT], f32, tag="negC0")
        ones_col = mem.tile([128, 1], f32, tag="ones_c")
        ones_row_r = mem.tile([1, 128], f32r, tag="ones_r")
        id1 = mem.tile([128, 128], f32, tag="id1")
        # phase 1/2 tiles on shared tags (slots reused by phase-5 scratch later)
        envT = mem.tile([128, NKT, ES], f32, tag="s8a")      # env^T shard [d, e_local]
        hT = mem.tile([128, NKT, ES], f32, tag="s8b")
        w1sb = mem.tile([128, NKT, D], f32, tag="s8c")
        w2sb = mem.tile([128, NKT, D], f32, tag="s8d")
        b1sb = mem.tile([128, NKT], f32, tag="b1")
        b2sb = mem.tile([128, NKT], f32, tag="b2")
        prjhi_sh = mem.tile([128, NKT, ES], f32r, tag="s4a")
        prjlo_sh = mem.tile([128, NKT, ES], f32r, tag="s4b")
        npn_hi = mem.tile([1, ES], f32r, tag="npnh")
        npn_lo = mem.tile([1, ES], f32r, tag="npnl")

        # ---- constants ----
        nc.gpsimd.memset(ones_col[:], 1.0)
        ones_row_f = work.tile([1, 128], f32, tag="ones_rf")
        nc.gpsimd.memset(ones_row_f[:], 1.0)
        nc.vector.tensor_copy(ones_row_r[:], ones_row_f[:])
        iorow = work.tile([128, 128], i32, tag="iorow")
        nc.gpsimd.iota(iorow[:], pattern=[[1, 128]], base=0, channel_multiplier=0)
        ioc = work.tile([128, 1], i32, tag="ioc")
        nc.gpsimd.iota(ioc[:], pattern=[[1, 1]], base=0, channel_multiplier=1)
        ioc_f = work.tile([128, 1], f32, tag="iocf")
        nc.vector.tensor_copy(ioc_f[:], ioc[:])
        nc.vector.tensor_scalar(id1[:], iorow[:], ioc_f[:], None, ALU.is_equal)

        # ---- loads ----
        nc.sync.dma_start(w1sb[:], w1_in.rearrange("(i p) m -> p i m", p=128))
        nc.sync.dma_start(w2sb[:], w2_in.rearrange("(i p) m -> p i m", p=128))
        nc.sync.dma_start(b1sb[:], b1_in[:])
        nc.sync.dma_start(b2sb[:], b2_in[:])
        nc.sync.dma_start(t_all[:], t_in[:])

        # ---- phase 1: env^T shard via PE transposes ----
        for r in range(ES // 128):
            envst = work.tile([128, D], f32, tag="envst")
            nc.sync.dma_start(envst[:], env_in[ts(r, 128), :])
            tp = ps_tp.tile([128, 512], f32, tag="tp")
            for i in range(NKT):
                nc.tensor.transpose(tp[:, ts(i, 128)], envst[:, ts(i, 128)], id1[:])
            nc.vector.tensor_copy(
                envT[:, :, ts(r, 128)],
                tp[:].rearrange("p (i e) -> p i e", i=NKT),
            )

        # ---- phase 2: sharded projection (native fp32 matmuls, exact) ----
        for j2 in range(NKT):
            ps2 = ps_tp.tile([128, 512], f32, tag="tp")
            for i in range(NKT):
                nc.tensor.matmul(
                    ps2[:, :ES], w1sb[:, i, ts(j2, 128)], envT[:, i, :],
                    start=(i == 0), stop=(i == NKT - 1),
                )
            nc.scalar.activation(hT[:, j2, :], ps2[:, :ES], AF.Relu, bias=b1sb[:, j2 : j2 + 1])
        pn_ps = ps_mm.tile([128, CHUNK], f32, tag="mm")
        for j2 in range(NKT):
            ps2 = ps_tp.tile([128, 512], f32, tag="tp")
            for i in range(NKT):
                nc.tensor.matmul(
                    ps2[:, :ES], w2sb[:, i, ts(j2, 128)], hT[:, i, :],
                    start=(i == 0), stop=(i == NKT - 1),
                )
            prj = work.tile([128, ES], f32, tag="prj")
            nc.scalar.activation(prj[:], ps2[:, :ES], AF.Identity, bias=b2sb[:, j2 : j2 + 1])
            nc.vector.tensor_copy(prjhi_sh[:, j2, :], prj[:])
            nc.vector.scalar_tensor_tensor(
                prjlo_sh[:, j2, :], prj[:], 1.0, prjhi_sh[:, j2, :],
                ALU.mult, ALU.subtract,
            )
            sqj = work.tile([128, ES], f32, tag="sqj")
            nc.scalar.activation(sqj[:], prj[:], AF.Square)
            nc.tensor.matmul(
                pn_ps[0:1, :ES], ones_col[:], sqj[:],
                start=(j2 == 0), stop=(j2 == NKT - 1),
            )
        nc.vector.tensor_scalar_mul(npn_hi[:], pn_ps[0:1, :ES], -1.0)
        nc.vector.scalar_tensor_tensor(
            npn_lo[:], pn_ps[0:1, :ES], -1.0, npn_hi[:], ALU.mult, ALU.subtract
        )

        # ---- phase 3: AllGather of [projT_hi; projT_lo; -pn_hi; -pn_lo] ----
        nc.sync.dma_start(ag_in[0:D, :].rearrange("(i p) e -> p i e", p=128), prjhi_sh[:])
        nc.sync.dma_start(ag_in[D : 2 * D, :].rearrange("(i p) e -> p i e", p=128), prjlo_sh[:])
        nc.sync.dma_start(ag_in[2 * D : 2 * D + 1, :], npn_hi[:])
        nc.sync.dma_start(ag_in[2 * D + 1 : 2 * D + 2, :], npn_lo[:])
        nc.gpsimd.collective_compute(
            "AllGather",
            ALU.bypass,
            replica_groups=[list(range(NCORES))],
            ins=[ag_in[:]],
            outs=[ag_out[:]],
        )

        # ---- phase 4: query^T (x2) via PE transposes, hi/lo split ----
        for j in range(NQT):
            qst = work.tile([128, D], f32, tag="qst")
            nc.sync.dma_start(qst[:], q_in[ts(j, 128), :])
            tpq = ps_tp.tile([128, 512], f32, tag="tp")
            for i in range(NKT):
                nc.tensor.transpose(tpq[:, ts(i, 128)], qst[:, ts(i, 128)], id1[:])
            nc.vector.tensor_scalar_mul(
                qT_hi[:, :, ts(j, 128)],
                tpq[:].rearrange("p (i q) -> p i q", i=NKT),
                2.0,
            )

        # phase-5 scratch on slots freed by phase 1/2 tiles
        iota_f = mem.tile([128, E], f32, tag="s8a")     # reuses envT slot
        xdump = mem.tile([128, CHUNK], f32, tag="s8b")  # ACT scratch (hT slot)
        idump = mem.tile([128, CHUNK], f32, tag="s8c")  # DVE scratch (w1sb slot)
        gdump = mem.tile([128, CHUNK], f32, tag="s8d")  # GpSimd scratch (w2sb slot)
        for c in range(NCH):
            ioe = work.tile([128, CHUNK], i32, tag="ioe")
            nc.gpsimd.iota(ioe[:], pattern=[[1, CHUNK]], base=c * CHUNK, channel_multiplier=0)
            nc.vector.tensor_copy(iota_f[:, ts(c, CHUNK)], ioe[:])

        # ---- phase 5: main loop over e-chunks x q-tiles ----
        for c in range(NCH):
            prjhi = proj_pool.tile([128, NKT, CHUNK], f32r, tag="prjhi")
            prjlo = proj_pool.tile([128, NKT, CHUNK], f32r, tag="prjlo")
            npnh = npn_pool.tile([1, CHUNK], f32r, tag="npnh")
            npnl = npn_pool.tile([1, CHUNK], f32r, tag="npnl")
            for h in range(CHUNK // 512):
                b = c * (CHUNK // 512) + h
                base = b * ROWS
                nc.sync.dma_start(
                    prjhi[:, :, ts(h, 512)],
                    ag_out[base : base + D, :].rearrange("(i p) e -> p i e", p=128),
                )
                nc.sync.dma_start(
                    prjlo[:, :, ts(h, 512)],
                    ag_out[base + D : base + 2 * D, :].rearrange("(i p) e -> p i e", p=128),
                )
                nc.sync.dma_start(
                    npnh[:, ts(h, 512)], ag_out[base + 2 * D : base + 2 * D + 1, :]
                )
                nc.sync.dma_start(
                    npnl[:, ts(h, 512)], ag_out[base + 2 * D + 1 : base + 2 * D + 2, :]
                )
            for j in range(NQT):
                ps = ps_mm.tile([128, CHUNK], f32, tag="mm")
                for h in range(CHUNK // 512):
                    sl = slice(h * 512, (h + 1) * 512)
                    for i in range(NKT):
                        nc.tensor.matmul(
                            ps[:, sl], qT_hi[:, i, ts(j, 128)], prjhi[:, i, sl],
                            start=(i == 0), stop=False,
                        )
                    for i in range(NKT):
                        nc.tensor.matmul(
                            ps[:, sl], qT_hi[:, i, ts(j, 128)], prjlo[:, i, sl],
                            start=False, stop=False,
                        )
                    nc.tensor.matmul(
                        ps[:, sl], ones_row_r[:], npnh[:, sl], start=False, stop=False
                    )
                    nc.tensor.matmul(
                        ps[:, sl], ones_row_r[:], npnl[:, sl], start=False, stop=True
                    )
                col = j * NCH + c
                mc = stats_m[:, col : col + 1]
                nc.vector.reduce_max(mc, ps[:], axis=AX.X)
                if c == 0:
                    # negC0 = -(chunk0 max) - 64*ln2; the extra shift keeps
                    # sum(exp(L + negC0)) inside the ACT Ln domain (max over a
                    # later chunk can exceed C0 by ~70)
                    nc.vector.tensor_scalar(
                        negC0[:, j : j + 1], mc, -1.0, -44.3614195558365,
                        ALU.mult, ALU.add,
                    )
                nc.scalar.activation(
                    xdump[:], ps[:], AF.Exp,
                    bias=negC0[:, j : j + 1],
                    accum_out=stats_s[:, col : col + 1],
                )
                nc.vector.scalar_tensor_tensor(
                    idump[:], ps[:], mc, iota_f[:, ts(c, CHUNK)],
                    ALU.is_equal, ALU.mult,
                    accum_out=stats_i[:, col : col + 1],
                )
                nc.vector.scalar_tensor_tensor(
                    gdump[:], iota_f[:, ts(c, CHUNK)], t_all[:, j : j + 1], ps[:],
                    ALU.is_equal, ALU.mult,
                    accum_out=stats_g[:, col : col + 1],
                )

        # ---- phase 6: finalize ----
        m_all = mem.tile([128, NQT], f32, tag="fin_m")
        idx_all = mem.tile([128, NQT], f32, tag="fin_i")
        g_all = mem.tile([128, NQT], f32, tag="fin_g")
        s_all = mem.tile([128, NQT], f32, tag="fin_s")
        lns = mem.tile([128, NQT], f32, tag="fin_ln")
        lse = mem.tile([128, NQT], f32, tag="fin_lse")
        lossv = mem.tile([128, NQT], f32, tag="fin_lv")
        corr = mem.tile([128, NQT], f32, tag="fin_co")
        la = mem.tile([128, 2], f32, tag="fin_la")
        pred_i = mem.tile([128, NQT], i32, tag="fin_pi")
        sc_sb = mem.tile([2, 1], f32, tag="fin_sc")

        for j in range(NQT):
            sl = slice(j * NCH, (j + 1) * NCH)
            nc.vector.reduce_max(m_all[:, j : j + 1], stats_m[:, sl], axis=AX.X)
            nc.vector.scalar_tensor_tensor(
                idump[:, 0:NCH], stats_m[:, sl], m_all[:, j : j + 1], stats_i[:, sl],
                ALU.is_equal, ALU.mult,
                accum_out=idx_all[:, j : j + 1],
            )
            nc.vector.reduce_sum(g_all[:, j : j + 1], stats_g[:, sl], axis=AX.X)
            nc.vector.reduce_sum(s_all[:, j : j + 1], stats_s[:, sl], axis=AX.X)
        nc.scalar.activation(lns[:], s_all[:], AF.Ln)
        nc.vector.tensor_sub(lse[:], lns[:], negC0[:])
        nc.vector.tensor_sub(lossv[:], lse[:], g_all[:])
        nc.vector.tensor_tensor(corr[:], idx_all[:], t_all[:], ALU.is_equal)
        nc.vector.reduce_sum(la[:, 0:1], lossv[:], axis=AX.X)
        nc.vector.reduce_sum(la[:, 1:2], corr[:], axis=AX.X)
        scps = ps_tp.tile([128, 512], f32, tag="tp")
        nc.tensor.matmul(scps[0:2, 0:1], la[:], ones_col[:], start=True, stop=True)
        nc.vector.tensor_copy(sc_sb[:], scps[0:2, 0:1])
        nc.sync.dma_start(part_out[:], sc_sb[:])
        nc.vector.tensor_copy(pred_i[:], idx_all[:])
        nc.sync.dma_start(pred_out[:], pred_i[:])
        if debug:
            for k, tl in enumerate((m_all, idx_all, g_all, s_all, lse, negC0)):
                nc.sync.dma_start(dbg_out[:, k * NQT : (k + 1) * NQT], tl[:])

    nc.compile()
    return nc


def _prep_inputs(environment_embeddings, query_embeddings, query_targets, w1, b1, w2, b2):
    env = np.ascontiguousarray(np.asarray(environment_embeddings, dtype=np.float32))
    q = np.ascontiguousarray(np.asarray(query_embeddings, dtype=np.float32))
    t = np.asarray(query_targets)
    w1 = np.ascontiguousarray(np.asarray(w1, dtype=np.float32))
    w2 = np.ascontiguousarray(np.asarray(w2, dtype=np.float32))
    b1t = np.ascontiguousarray(np.asarray(b1, dtype=np.float32).reshape(NKT, 128).T)
    b2t = np.ascontiguousarray(np.asarray(b2, dtype=np.float32).reshape(NKT, 128).T)
    t_f = t.astype(np.float32)
    in_maps = []
    for c in range(NCORES):
        in_maps.append(
            {
                "q_shard": q[c * QS : (c + 1) * QS],
                "env_shard": env[c * ES : (c + 1) * ES],
                "w1": w1,
                "w2": w2,
                "b1t": b1t,
                "b2t": b2t,
                "t_shard": np.ascontiguousarray(
                    t_f[c * QS : (c + 1) * QS].reshape(NQT, 128).T
                ),
            }
        )
    return in_maps, t


def _combine(results, targets_dtype):
    preds = np.concatenate([np.asarray(r["pred"]).T.reshape(-1) for r in results])
    loss_sum = sum(float(np.asarray(r["partials"])[0, 0]) for r in results)
    acc_sum = sum(float(np.asarray(r["partials"])[1, 0]) for r in results)
    loss = np.float32(loss_sum / Q)
    acc = np.float32(acc_sum / Q)
    idx_dtype = np.int64 if np.dtype(targets_dtype) == np.int64 else np.int32
    return preds.astype(idx_dtype), loss, acc


def kernel(environment_embeddings, query_embeddings, query_targets, w1, b1, w2, b2):
    from concourse.bass_utils import run_bass_kernel_spmd

    if "nc" not in _CACHE:
        _CACHE["nc"] = _build()
    nc = _CACHE["nc"]
    in_maps, t = _prep_inputs(
        environment_embeddings, query_embeddings, query_targets, w1, b1, w2, b2
    )
    res = run_bass_kernel_spmd(nc, in_maps, list(range(NCORES)))
    return _combine(res.results, t.dtype)
